# revision 3
# baseline (speedup 1.0000x reference)
"""Multi-head attention on 8 Trainium2 NeuronCores.

Problem: B=2, L=2048, H=1024, N=16 heads, D=64 (dense_transformer).

Sharding: core c handles batch b = c//4 and head group g = c%4 (4 heads
each).  Every core computes Q/K/V projections for its 4 heads, the full
softmax(QK^T/sqrt(D))V for its heads, and a partial output projection
(contraction over its heads' (n,d) slice of wo).  The 4 partial outputs
per batch are summed on the host — no cross-core collectives needed.

Layout tricks (all transposes are free, done on host or by choosing the
matmul operand order):
  - activations are fed transposed (x^T: [H, L]) so projections produce
    Q^T/K^T ([D, L], head-pair packed on the partition dim) directly.
  - scores are computed transposed (S^T: [t, f]) so the attention matmul
    consumes exp(S^T) as the moving operand with contraction over t.
  - V is produced in natural [t, d] layout by swapping matmul operands.
  - a ones-column appended to V yields the softmax denominators as row 64
    of the attention output (no separate reduction).
  - K=64 score matmuls are row-tiled: the two heads of a pair occupy PE
    array rows 0-63 / 64-127 and run concurrently.
Compute in bf16 (fp32 PSUM accumulation), output fp32.
"""

import numpy as np
import ml_dtypes

B, L, H, NH, DH = 2, 2048, 1024, 16, 64
P = 128
HC = H // P           # 8 h-chunks of 128
TC = L // P           # 16 t-chunks of 128
FB = L // 512         # 4 f-blocks of 512
NCORES = 8
HEADS_PER_CORE = 4    # 2 pairs
BF16 = ml_dtypes.bfloat16

_CACHE = {}


def _build(with_bias: bool):
    import concourse.mybir as mybir
    import concourse.tile as tile
    from concourse import bacc

    f32 = mybir.dt.float32
    bf16 = mybir.dt.bfloat16
    Exp = mybir.ActivationFunctionType.Exp

    nc = bacc.Bacc("TRN2", target_bir_lowering=False, debug=False,
                   num_devices=NCORES)

    xq_d = nc.dram_tensor("xq_t", [HC, P, L], bf16, kind="ExternalInput")
    xs_d = nc.dram_tensor("xs_t", [HC, P, L], bf16, kind="ExternalInput")
    wq_d = nc.dram_tensor("wq_pk", [2, HC, P, 128], bf16, kind="ExternalInput")
    wk_d = nc.dram_tensor("wk_pk", [2, HC, P, 128], bf16, kind="ExternalInput")
    wv_d = nc.dram_tensor("wv_pk", [HC, P, 256], bf16, kind="ExternalInput")
    wo_d = nc.dram_tensor("wo_pk", [2, P, H], bf16, kind="ExternalInput")
    if with_bias:
        bias_d = nc.dram_tensor("bias_t", [TC, P, L], bf16, kind="ExternalInput")
    out_d = nc.dram_tensor("out", [HC, P, L], f32, kind="ExternalOutput")

    with tile.TileContext(nc) as tc:
        with (
            tc.tile_pool(name="const", bufs=1) as const,
            tc.tile_pool(name="acts", bufs=1) as acts,
            tc.tile_pool(name="ps", bufs=2, space="PSUM") as ps_pool,
            tc.tile_pool(name="a_ps", bufs=2, space="PSUM") as a_ps,
            tc.tile_pool(name="exp_sb", bufs=6) as exp_pool,
            tc.tile_pool(name="small", bufs=2) as small,
            tc.tile_pool(name="evict", bufs=3) as evict,
            tc.tile_pool(name="bias_sb", bufs=3) as bias_pool,
        ):
            # ---- resident SBUF tensors ----
            xq_sb = acts.tile([P, HC, L], bf16)
            xs_sb = acts.tile([P, HC, L], bf16)
            wq_sb = const.tile([P, 2, HC, 128], bf16)
            wk_sb = const.tile([P, 2, HC, 128], bf16)
            wv_sb = const.tile([P, HC, 256], bf16)
            wo_sb = const.tile([P, 2, H], bf16)
            qt_sb = acts.tile([P, 2, L], bf16)   # Q^T, head-pair packed
            kt_sb = acts.tile([P, 2, L], bf16)   # K^T, head-pair packed
            v_sb = acts.tile([P, TC, HEADS_PER_CORE, 65], bf16)  # V || ones
            att_sb = acts.tile([P, 2, FB, 512], bf16)  # normalized attn^T

            # ---- input DMAs ----
            for hc in range(HC):
                nc.sync.dma_start(xq_sb[:, hc, :], xq_d.ap()[hc])
                nc.sync.dma_start(xs_sb[:, hc, :], xs_d.ap()[hc])
            for pp in range(2):
                for hc in range(HC):
                    nc.sync.dma_start(wq_sb[:, pp, hc, :], wq_d.ap()[pp, hc])
                    nc.sync.dma_start(wk_sb[:, pp, hc, :], wk_d.ap()[pp, hc])
            for hc in range(HC):
                nc.sync.dma_start(wv_sb[:, hc, :], wv_d.ap()[hc])
            for pp in range(2):
                nc.sync.dma_start(wo_sb[:, pp, :], wo_d.ap()[pp])

            nc.vector.memset(v_sb[:, :, :, 64:65], 1.0)

            # ---- projections ----
            def project(pair, src_sb, w_sb, dst_sb):
                # dst^T[2-head-pack, L] = (w_pair)^T @ x^T
                for fb in range(FB):
                    ps = ps_pool.tile([P, 1024], f32, tag="s", name="proj_ps")
                    for hc in range(HC):
                        nc.tensor.matmul(
                            ps[:, 0:512],
                            lhsT=w_sb[:, pair, hc, :],
                            rhs=src_sb[:, hc, fb * 512:(fb + 1) * 512],
                            start=(hc == 0), stop=(hc == HC - 1),
                        )
                    nc.vector.tensor_copy(
                        dst_sb[:, pair, fb * 512:(fb + 1) * 512], ps[:, 0:512])

            project(0, xq_sb, wq_sb, qt_sb)
            project(0, xs_sb, wk_sb, kt_sb)

            # V[t, n*64+d] for all 4 local heads (operand-swapped matmul)
            for t in range(TC):
                vp = ps_pool.tile([P, 1024], f32, tag="s", name="v_ps")
                for hc in range(HC):
                    nc.tensor.matmul(
                        vp[:, 0:256],
                        lhsT=xs_sb[:, hc, t * P:(t + 1) * P],
                        rhs=wv_sb[:, hc, :],
                        start=(hc == 0), stop=(hc == HC - 1),
                    )
                nc.vector.tensor_copy(v_sb[:, t, :, 0:64], vp[:, 0:256])

            project(1, xq_sb, wq_sb, qt_sb)
            project(1, xs_sb, wk_sb, kt_sb)

            if with_bias:
                bias_tiles = []
                for t in range(TC):
                    bt = bias_pool.tile([P, L], bf16, tag="bias")
                    nc.sync.dma_start(bt[:], bias_d.ap()[t])
                    bias_tiles.append(bt)

            # ---- attention + output projection ----
            for half in range(2):
                for pair in range(2):
                    at = {j: a_ps.tile([P, 1024], f32, tag="a", name=f"at{j}")
                          for j in (0, 1)}
                    for t in range(TC):
                        es = {}
                        for j in (0, 1):
                            base = j * 64
                            s = ps_pool.tile([P, 1024], f32, tag="s", name="s_ps")
                            for sl in (0, 1):
                                fcol = (half * 2 + sl) * 512
                                nc.tensor.matmul(
                                    s[:, sl * 512:(sl + 1) * 512],
                                    lhsT=kt_sb[base:base + 64, pair,
                                               t * P:(t + 1) * P],
                                    rhs=qt_sb[base:base + 64, pair,
                                              fcol:fcol + 512],
                                    start=True, stop=True,
                                )
                            if with_bias:
                                nc.vector.tensor_add(
                                    s[:], s[:],
                                    bias_tiles[t][:, half * 1024:
                                                  (half + 1) * 1024])
                            e = exp_pool.tile([P, 1024], bf16, tag="e")
                            nc.scalar.activation(e[:], s[:], Exp, scale=0.125)
                            es[j] = e
                        for j in (0, 1):
                            for sl in (0, 1):
                                nc.tensor.matmul(
                                    at[j][0:65, sl * 512:(sl + 1) * 512],
                                    lhsT=v_sb[:, t, 2 * pair + j, :],
                                    rhs=es[j][:, sl * 512:(sl + 1) * 512],
                                    start=(t == 0), stop=(t == TC - 1),
                                )
                    # normalize by softmax denominator (row 64)
                    for j in (0, 1):
                        rec = small.tile([1, 1024], f32, tag="rec")
                        nc.vector.reciprocal(rec[:], at[j][64:65, :])
                        recb = small.tile([64, 1024], f32, tag="recb")
                        nc.gpsimd.partition_broadcast(recb[:], rec[:],
                                                      channels=64)
                        nc.vector.tensor_mul(
                            att_sb[j * 64:(j + 1) * 64, pair,
                                   half * 2:half * 2 + 2, :],
                            at[j][0:64, :], recb[:])

                # partial output projection for this half's two f-blocks
                for sl in (0, 1):
                    fb = half * 2 + sl
                    for ht in range(HC):
                        op = ps_pool.tile([P, 1024], f32, tag="s", name="o_ps")
                        for pair in range(2):
                            nc.tensor.matmul(
                                op[:, 0:512],
                                lhsT=wo_sb[:, pair, ht * P:(ht + 1) * P],
                                rhs=att_sb[:, pair, fb, :],
                                start=(pair == 0), stop=(pair == 1),
                            )
                        ot = evict.tile([P, 512], f32, tag="ot")
                        nc.vector.tensor_copy(ot[:], op[:, 0:512])
                        nc.sync.dma_start(
                            out_d.ap()[ht, :, fb * 512:(fb + 1) * 512], ot[:])

    nc.compile()
    return nc


def _get_nc(with_bias: bool):
    key = ("nc", with_bias)
    if key not in _CACHE:
        _CACHE[key] = _build(with_bias)
    return _CACHE[key]


def kernel(query_input, source_input, bias, wq, wk, wv, wo):
    from concourse.bass_utils import run_bass_kernel_spmd

    query_input = np.asarray(query_input)
    source_input = np.asarray(source_input)
    bias = np.asarray(bias)
    wq, wk, wv, wo = (np.asarray(a) for a in (wq, wk, wv, wo))

    with_bias = bool(np.any(bias))
    nc = _get_nc(with_bias)

    # host-side prep: transpose + cast + pack per core
    xq_t = [np.ascontiguousarray(query_input[b].T).astype(BF16)
            .reshape(HC, P, L) for b in range(B)]
    xs_t = [np.ascontiguousarray(source_input[b].T).astype(BF16)
            .reshape(HC, P, L) for b in range(B)]
    if with_bias:
        bias_t = [np.ascontiguousarray(bias[b, 0].T * 8.0).astype(BF16)
                  .reshape(TC, P, L) for b in range(B)]

    in_maps = []
    for c in range(NCORES):
        b, g = divmod(c, HEADS_PER_CORE)
        n0 = g * HEADS_PER_CORE
        wq_pk = np.stack([
            np.ascontiguousarray(
                wq[:, n0 + 2 * pp:n0 + 2 * pp + 2, :].reshape(HC, P, 128))
            for pp in range(2)]).astype(BF16)
        wk_pk = np.stack([
            np.ascontiguousarray(
                wk[:, n0 + 2 * pp:n0 + 2 * pp + 2, :].reshape(HC, P, 128))
            for pp in range(2)]).astype(BF16)
        wv_pk = np.ascontiguousarray(
            wv[:, n0:n0 + HEADS_PER_CORE, :].reshape(HC, P, 256)).astype(BF16)
        wo_pk = np.ascontiguousarray(
            wo[n0:n0 + HEADS_PER_CORE].reshape(2, P, H)).astype(BF16)
        m = {"xq_t": xq_t[b], "xs_t": xs_t[b],
             "wq_pk": wq_pk, "wk_pk": wk_pk, "wv_pk": wv_pk, "wo_pk": wo_pk}
        if with_bias:
            m["bias_t"] = bias_t[b]
        in_maps.append(m)

    res = run_bass_kernel_spmd(nc, in_maps, core_ids=list(range(NCORES)))

    out = np.zeros((B, L, H), np.float32)
    for c in range(NCORES):
        b = c // HEADS_PER_CORE
        out[b] += res.results[c]["out"].reshape(H, L).T
    return out


# revision 5
# speedup vs baseline: 1.0971x; 1.0971x over previous
"""Multi-head attention on 8 Trainium2 NeuronCores.

Problem: B=2, L=2048, H=1024, N=16 heads, D=64 (dense_transformer).

Sharding: core c handles batch b = c//4 and head group g = c%4 (4 heads
each).  Every core computes Q/K/V projections for its 4 heads, the full
softmax(QK^T/sqrt(D))V for its heads, and a partial output projection
(contraction over its heads' (n,d) slice of wo).  The 4 partial outputs
per batch are summed on the host — no cross-core collectives needed.

Layout tricks (all transposes are free, done on host or by choosing the
matmul operand order):
  - activations are fed transposed (x^T: [H, L]) so projections produce
    Q^T/K^T ([D, L], head-pair packed on the partition dim) directly.
  - scores are computed transposed (S^T: [t, f]) so the attention matmul
    consumes exp(S^T) as the moving operand with contraction over t.
  - V is produced in natural [t, d] layout by swapping matmul operands.
  - a ones-column appended to V yields the softmax denominators as row 64
    of the attention output (no separate reduction).
  - K=64 score matmuls are row-tiled: the two heads of a pair occupy PE
    array rows 0-63 / 64-127 and run concurrently.
Compute in bf16 (fp32 PSUM accumulation), output fp32.
"""

import numpy as np
import ml_dtypes

B, L, H, NH, DH = 2, 2048, 1024, 16, 64
P = 128
HC = H // P           # 8 h-chunks of 128
TC = L // P           # 16 t-chunks of 128
FB = L // 512         # 4 f-blocks of 512
NCORES = 8
HEADS_PER_CORE = 4    # 2 pairs
BF16 = ml_dtypes.bfloat16

_CACHE = {}


def _build(with_bias: bool):
    import concourse.mybir as mybir
    import concourse.tile as tile
    from concourse import bacc

    f32 = mybir.dt.float32
    bf16 = mybir.dt.bfloat16
    Exp = mybir.ActivationFunctionType.Exp

    nc = bacc.Bacc("TRN2", target_bir_lowering=False, debug=False,
                   num_devices=NCORES)

    xq_d = nc.dram_tensor("xq_t", [HC, P, L], bf16, kind="ExternalInput")
    xs_d = nc.dram_tensor("xs_t", [HC, P, L], bf16, kind="ExternalInput")
    wq_d = nc.dram_tensor("wq_pk", [2, HC, P, 128], bf16, kind="ExternalInput")
    wk_d = nc.dram_tensor("wk_pk", [2, HC, P, 128], bf16, kind="ExternalInput")
    wv_d = nc.dram_tensor("wv_pk", [HC, P, 256], bf16, kind="ExternalInput")
    wo_d = nc.dram_tensor("wo_pk", [2, P, H], bf16, kind="ExternalInput")
    if with_bias:
        bias_d = nc.dram_tensor("bias_t", [TC, P, L], bf16, kind="ExternalInput")
    out_d = nc.dram_tensor("out", [HC, P, L], f32, kind="ExternalOutput")

    with tile.TileContext(nc) as tc:
        with (
            tc.tile_pool(name="const", bufs=1) as const,
            tc.tile_pool(name="acts", bufs=1) as acts,
            tc.tile_pool(name="ps", bufs=2, space="PSUM") as ps_pool,
            tc.tile_pool(name="a_ps", bufs=2, space="PSUM") as a_ps,
            tc.tile_pool(name="exp_sb", bufs=6) as exp_pool,
            tc.tile_pool(name="small", bufs=2) as small,
            tc.tile_pool(name="evict", bufs=3) as evict,
            tc.tile_pool(name="bias_sb", bufs=3) as bias_pool,
        ):
            # ---- resident SBUF tensors ----
            xq_sb = acts.tile([P, HC, L], bf16)
            xs_sb = acts.tile([P, HC, L], bf16)
            wq_sb = const.tile([P, 2, HC, 128], bf16)
            wk_sb = const.tile([P, 2, HC, 128], bf16)
            wv_sb = const.tile([P, HC, 256], bf16)
            wo_sb = const.tile([P, 2, H], bf16)
            qt_sb = acts.tile([P, 2, L], bf16)   # Q^T, head-pair packed
            kt_sb = acts.tile([P, 2, L], bf16)   # K^T, head-pair packed
            v_sb = acts.tile([P, TC, HEADS_PER_CORE, P], bf16)  # V || ones x64
            att_sb = acts.tile([P, 2, FB, 512], bf16)  # normalized attn^T

            # ---- input DMAs ----
            for hc in range(HC):
                nc.sync.dma_start(xq_sb[:, hc, :], xq_d.ap()[hc])
                nc.sync.dma_start(xs_sb[:, hc, :], xs_d.ap()[hc])
            for pp in range(2):
                for hc in range(HC):
                    nc.sync.dma_start(wq_sb[:, pp, hc, :], wq_d.ap()[pp, hc])
                    nc.sync.dma_start(wk_sb[:, pp, hc, :], wk_d.ap()[pp, hc])
            for hc in range(HC):
                nc.sync.dma_start(wv_sb[:, hc, :], wv_d.ap()[hc])
            for pp in range(2):
                nc.sync.dma_start(wo_sb[:, pp, :], wo_d.ap()[pp])

            nc.vector.memset(v_sb[:, :, :, 64:128], 1.0)

            # ---- projections ----
            def project(pair, src_sb, w_sb, dst_sb):
                # dst^T[2-head-pack, L] = (w_pair)^T @ x^T
                for fb in range(FB):
                    ps = ps_pool.tile([P, 1024], f32, tag="s", name="proj_ps")
                    for hc in range(HC):
                        nc.tensor.matmul(
                            ps[:, 0:512],
                            lhsT=w_sb[:, pair, hc, :],
                            rhs=src_sb[:, hc, fb * 512:(fb + 1) * 512],
                            start=(hc == 0), stop=(hc == HC - 1),
                        )
                    nc.vector.tensor_copy(
                        dst_sb[:, pair, fb * 512:(fb + 1) * 512], ps[:, 0:512])

            project(0, xq_sb, wq_sb, qt_sb)
            project(0, xs_sb, wk_sb, kt_sb)

            # V[t, n*64+d] for all 4 local heads (operand-swapped matmul)
            for t in range(TC):
                vp = ps_pool.tile([P, 1024], f32, tag="s", name="v_ps")
                for hc in range(HC):
                    nc.tensor.matmul(
                        vp[:, 0:256],
                        lhsT=xs_sb[:, hc, t * P:(t + 1) * P],
                        rhs=wv_sb[:, hc, :],
                        start=(hc == 0), stop=(hc == HC - 1),
                    )
                nc.vector.tensor_copy(v_sb[:, t, :, 0:64], vp[:, 0:256])

            project(1, xq_sb, wq_sb, qt_sb)
            project(1, xs_sb, wk_sb, kt_sb)

            if with_bias:
                bias_tiles = []
                for t in range(TC):
                    bt = bias_pool.tile([P, L], bf16, tag="bias")
                    nc.sync.dma_start(bt[:], bias_d.ap()[t])
                    bias_tiles.append(bt)

            # ---- attention + output projection ----
            for half in range(2):
                for pair in range(2):
                    at = {j: a_ps.tile([P, 1024], f32, tag="a", name=f"at{j}")
                          for j in (0, 1)}
                    for t in range(TC):
                        es = {}
                        for j in (0, 1):
                            base = j * 64
                            s = ps_pool.tile([P, 1024], f32, tag="s", name="s_ps")
                            for sl in (0, 1):
                                fcol = (half * 2 + sl) * 512
                                nc.tensor.matmul(
                                    s[:, sl * 512:(sl + 1) * 512],
                                    lhsT=kt_sb[base:base + 64, pair,
                                               t * P:(t + 1) * P],
                                    rhs=qt_sb[base:base + 64, pair,
                                              fcol:fcol + 512],
                                    start=True, stop=True,
                                )
                            if with_bias:
                                nc.vector.tensor_add(
                                    s[:], s[:],
                                    bias_tiles[t][:, half * 1024:
                                                  (half + 1) * 1024])
                            e = exp_pool.tile([P, 1024], bf16, tag="e")
                            nc.scalar.activation(e[:], s[:], Exp, scale=0.125)
                            es[j] = e
                        for j in (0, 1):
                            for sl in (0, 1):
                                nc.tensor.matmul(
                                    at[j][:, sl * 512:(sl + 1) * 512],
                                    lhsT=v_sb[:, t, 2 * pair + j, :],
                                    rhs=es[j][:, sl * 512:(sl + 1) * 512],
                                    start=(t == 0), stop=(t == TC - 1),
                                )
                    # Normalize by the softmax denominators (the 64
                    # ones-columns of V make rows 64:128 of the accumulator
                    # 64 identical denominator rows).  Engine ops must not
                    # move data across partitions with single-partition APs
                    # (HW misreads); this chain only uses 32-aligned
                    # 64-partition blocks, which hardware handles.
                    for j in (0, 1):
                        den = small.tile([64, 1024], f32, tag="den")
                        nc.vector.tensor_copy(den[:], at[j][64:128, :])
                        recb = small.tile([64, 1024], f32, tag="recb")
                        nc.vector.reciprocal_approx_fast(recb[:], den[:])
                        nc.vector.tensor_mul(
                            att_sb[j * 64:(j + 1) * 64, pair,
                                   half * 2:half * 2 + 2, :],
                            at[j][0:64, :], recb[:])

                # partial output projection for this half's two f-blocks
                for sl in (0, 1):
                    fb = half * 2 + sl
                    for ht in range(HC):
                        op = ps_pool.tile([P, 1024], f32, tag="s", name="o_ps")
                        for pair in range(2):
                            nc.tensor.matmul(
                                op[:, 0:512],
                                lhsT=wo_sb[:, pair, ht * P:(ht + 1) * P],
                                rhs=att_sb[:, pair, fb, :],
                                start=(pair == 0), stop=(pair == 1),
                            )
                        ot = evict.tile([P, 512], f32, tag="ot")
                        nc.vector.tensor_copy(ot[:], op[:, 0:512])
                        nc.sync.dma_start(
                            out_d.ap()[ht, :, fb * 512:(fb + 1) * 512], ot[:])

    nc.compile()
    return nc


def _get_nc(with_bias: bool):
    key = ("nc", with_bias)
    if key not in _CACHE:
        _CACHE[key] = _build(with_bias)
    return _CACHE[key]


def kernel(query_input, source_input, bias, wq, wk, wv, wo):
    from concourse.bass_utils import run_bass_kernel_spmd

    query_input = np.asarray(query_input)
    source_input = np.asarray(source_input)
    bias = np.asarray(bias)
    wq, wk, wv, wo = (np.asarray(a) for a in (wq, wk, wv, wo))

    with_bias = bool(np.any(bias))
    nc = _get_nc(with_bias)

    # host-side prep: transpose + cast + pack per core
    xq_t = [np.ascontiguousarray(query_input[b].T).astype(BF16)
            .reshape(HC, P, L) for b in range(B)]
    xs_t = [np.ascontiguousarray(source_input[b].T).astype(BF16)
            .reshape(HC, P, L) for b in range(B)]
    if with_bias:
        bias_t = [np.ascontiguousarray(bias[b, 0].T * 8.0).astype(BF16)
                  .reshape(TC, P, L) for b in range(B)]

    in_maps = []
    for c in range(NCORES):
        b, g = divmod(c, HEADS_PER_CORE)
        n0 = g * HEADS_PER_CORE
        wq_pk = np.stack([
            np.ascontiguousarray(
                wq[:, n0 + 2 * pp:n0 + 2 * pp + 2, :].reshape(HC, P, 128))
            for pp in range(2)]).astype(BF16)
        wk_pk = np.stack([
            np.ascontiguousarray(
                wk[:, n0 + 2 * pp:n0 + 2 * pp + 2, :].reshape(HC, P, 128))
            for pp in range(2)]).astype(BF16)
        wv_pk = np.ascontiguousarray(
            wv[:, n0:n0 + HEADS_PER_CORE, :].reshape(HC, P, 256)).astype(BF16)
        wo_pk = np.ascontiguousarray(
            wo[n0:n0 + HEADS_PER_CORE].reshape(2, P, H)).astype(BF16)
        m = {"xq_t": xq_t[b], "xs_t": xs_t[b],
             "wq_pk": wq_pk, "wk_pk": wk_pk, "wv_pk": wv_pk, "wo_pk": wo_pk}
        if with_bias:
            m["bias_t"] = bias_t[b]
        in_maps.append(m)

    res = run_bass_kernel_spmd(nc, in_maps, core_ids=list(range(NCORES)))

    out = np.zeros((B, L, H), np.float32)
    for c in range(NCORES):
        b = c // HEADS_PER_CORE
        out[b] += res.results[c]["out"].reshape(H, L).T
    return out
